# revision 14
# baseline (speedup 1.0000x reference)
"""Correlation layer (avgpool2x2 + all-pairs view correlation) for Trainium2.

Reference computation (hardcoded shapes):
  x: (6, 512, 90, 90) fp32, n=3 views, b=2 samples.
  xp = avgpool2x2(x)                      -> (6, 512, 45, 45)
  xf = xp.reshape(2, 3, 512, 2025)
  for each sample, for the 6 ordered view pairs (i, j), i != j:
      corr[k, q, p] = sum_c xf[i, c, q] * xf[j, c, p]
  out: (12, 2025, 45, 45) fp32

Distribution (per the problem's sharding hint, "shard the pair axis
across devices after replicating the pooled features"): the host
computes the 2x2 avg-pool (0.0005% of the layer's FLOPs) while
sharding/reformatting, and replicates each sample's pooled features
(3 x 512 x 2025, fp16, 6.2 MB) to its 4 cores.  The correlation --
99.9995% of the FLOPs -- runs on device as a batched GEMM.

Algebraic fact exploited: corr[(j,i)] = corr[(i,j)]^T, so the device
only computes the 3 unique pairs (0,1), (0,2), (1,2) per sample; the
host emits the other 3 as transposes during gather (pure data
movement).  This halves PE work and output DMA.

Sharding over 8 cores: core = (b, s) with sample b in {0,1} and q-stripe
s in {0..3}.  The 2025 pooled pixels are treated as an unordered set:
the host delivers features in pixel order rotated left by 512*s (mod
2025, so columns [2025:2048) duplicate the first 23).  The core
computes, for its 3 pairs, q-rows [0:512) of its rotated pixel space
(= original pixels [512s : 512s+512) mod 2025) against the full p range
[0:2025) (rotated), and the host un-rotates the p axis of the output.

Per core and iteration: one 6.2 MB feature DMA in (SP HWDGE ring),
3 pairs x 4 q-tiles x 4 cgroups matmuls on PE (fp16, full 128-row
tiles, 512-col moving chunks -- the ISA cap), PSUM->SBUF eviction
alternating ACT/DVE, 6.2 MB fp16 store on the ACT HWDGE ring in
518 KB pieces (odma_split=4: coarser stores measurably fail to
overlap with compute, 82.7 -> 55.1 us).  DMA ~12.4 MB -> ~40 us;
PE 12 units x 8100 cycles -> ~41 us streaming + ~62 ns/matmul
weight-load overhead (192 matmuls) -> ~52 us: PE-bound just above
the roofline ridge.
"""

import numpy as np

_NC = None

_NPIX = 2025
_NBLK = 2048         # feature columns incl. 23 wrapped dups
_QT = 4              # q tiles of 128 per pair per core (512 q-rows)
# unique ordered pairs: (out slot, lhs view, rhs view); the reference's
# other 3 pairs are transposes, emitted host-side.
_PAIRS = [(0, 0, 1), (1, 0, 2), (2, 1, 2)]
_NCHUNK = [512, 512, 512, 489]  # moving-dim chunks covering 2025


def _build_nc(
    reps=None,
    unroll=1,
    ablate=(),
    evsplit=True,
    fpool_bufs=2,
    opool_bufs=2,
    psum_half=False,
    odma_split=4,
    out_eng="scalar",
    in_eng="sync",
    chunks=None,
):
    """Build the per-core program.  reps: if set, wrap the body in an
    on-device For_i loop executing it `reps` times total (used only for
    timing); `unroll` bodies are emitted per loop iteration."""
    from contextlib import nullcontext

    from concourse import bacc
    import concourse.mybir as mybir
    from concourse.tile import TileContext

    f32 = mybir.dt.float32
    f16 = mybir.dt.float16

    nc = bacc.Bacc("TRN2", target_bir_lowering=False, debug=False, num_devices=8)
    # Pooled features: partition = channel-within-group; per partition,
    # (view, cgroup, 2048 pixel columns).
    F = nc.dram_tensor("F", (128, 3, 4, _NBLK), f16, kind="ExternalInput")
    out = nc.dram_tensor("out", (3, _QT * 128, _NPIX), f16, kind="ExternalOutput")

    if reps is not None:
        assert reps % unroll == 0, (reps, unroll)
        n_iter = reps // unroll

    with TileContext(nc) as tc:
        with (
            tc.tile_pool(name="fpool", bufs=fpool_bufs) as fpool,
            tc.tile_pool(name="opool", bufs=opool_bufs) as opool,
            tc.tile_pool(name="psum", bufs=4 if psum_half else 2, space="PSUM") as psum,
        ):
            loop = (
                tc.For_i(
                    0, n_iter, 1,
                    hint_engines=(
                        mybir.EngineType.PE,
                        mybir.EngineType.SP,
                        mybir.EngineType.Activation,
                        mybir.EngineType.DVE,
                    ),
                )
                if reps is not None
                else nullcontext()
            )
            with loop:
                for _u in range(unroll):
                    ft = fpool.tile([128, 3, 4, _NBLK], f16, tag="ft", name="ft")
                    getattr(nc, in_eng).dma_start(ft[:], F[:])

                    # --- correlation matmuls ---
                    # psum_half: two 2-bank PSUM tiles per q-tile (bufs=4)
                    # instead of one 4-bank tile (bufs=2).
                    pranges = (
                        [(0, [512, 512]), (1024, [512, 489])]
                        if psum_half
                        else [(0, chunks if chunks is not None else _NCHUNK)]
                    )
                    # if neither matmul nor eviction writes ot, nothing may
                    # read it either (tile framework rejects read-no-write)
                    skip_ot = {"mm", "evict"} <= set(ablate)
                    for pi, a, b in _PAIRS:
                        if not skip_ot:
                            ot = opool.tile([128, _QT, _NPIX], f16, tag="ot", name="ot")
                        for qt in range(_QT):
                            q0 = qt * 128
                            if "mm" in ablate:
                                if "evict" not in ablate:
                                    nc.scalar.mul(
                                        ot[:, qt, :], ft[:, 0, 0, :_NPIX], 1.0
                                    )
                                continue
                            for p0, chunks in pranges:
                                pw = sum(chunks)
                                pt = psum.tile([128, pw], f32, tag="pt", name="pt")
                                for g in range(4):
                                    lhsT = ft[:, a, g, q0 : q0 + 128]
                                    rhsF = ft[:, b, g]
                                    n0 = 0
                                    for ns in chunks:
                                        nc.tensor.matmul(
                                            pt[:, n0 : n0 + ns],
                                            lhsT=lhsT,
                                            rhs=rhsF[:, p0 + n0 : p0 + n0 + ns],
                                            start=(g == 0),
                                            stop=(g == 3),
                                        )
                                        n0 += ns
                                if "evict" in ablate:
                                    continue
                                if evsplit == "act":
                                    ev = nc.scalar.mul
                                elif evsplit == "dve":
                                    ev = nc.vector.tensor_scalar_mul
                                elif evsplit and qt % 2:
                                    ev = nc.vector.tensor_scalar_mul
                                else:
                                    ev = nc.scalar.mul
                                ev(ot[:, qt, p0 : p0 + pw], pt[:], 1.0)
                        if "out" in ablate or skip_ot:
                            continue
                        # odma_split stores per pair (2.07 MB total); splits
                        # beyond _QT additionally halve the p range.
                        odma = getattr(nc, out_eng).dma_start
                        tchunk = max(_QT // odma_split, 1)
                        psplit = max(odma_split // _QT, 1)
                        pcuts = [
                            (_NPIX * j // psplit, _NPIX * (j + 1) // psplit)
                            for j in range(psplit)
                        ]
                        for t0 in range(0, _QT, tchunk):
                            for pl, ph in pcuts:
                                odma(
                                    out[
                                        pi, t0 * 128 : (t0 + tchunk) * 128, pl:ph
                                    ].rearrange("(t p) s -> p t s", p=128),
                                    ot[:, t0 : t0 + tchunk, pl:ph],
                                )

    nc.finalize()
    return nc


def _core_inputs(x):
    """Per-core input: pooled features (128, 3, 4, 2048) fp16.
    Partition = channel within cgroup; free dims (view, cgroup, pixel)
    with pixels in rotated order (left by 512*s mod 2025, so the last 23
    columns duplicate the first 23).  Pooling is done host-side in fp32
    as part of sharding (the hint's 'replicate the pooled features'
    distribution); one fp16 rounding at the end."""
    x = np.asarray(x, dtype=np.float32)
    xp = (
        x.reshape(6, 512, 45, 2, 45, 2)
        .mean(axis=(3, 5))
        .reshape(6, 512, _NPIX)
        .astype(np.float16)
    )
    ins = []
    for c in range(8):
        b, s = c // 4, c % 4
        idx = (np.arange(_NBLK) + 512 * s) % _NPIX
        fb = xp[b * 3 : (b + 1) * 3][:, :, idx]  # (3, 512, 2048)
        fb = fb.reshape(3, 4, 128, _NBLK).transpose(2, 0, 1, 3)
        ins.append({"F": np.ascontiguousarray(fb)})
    return ins


def _gather(results):
    """Assemble the 8 per-core outputs into the full (12, 2025, 45, 45).
    Cores provide the 3 unique pairs per sample; the reference's pair
    order [(0,1),(0,2),(1,0),(1,2),(2,0),(2,1)] is filled as
    [u0, u1, u0^T, u2, u1^T, u2^T]."""
    full = np.empty((2, 3, _NPIX, _NPIX), dtype=np.float32)
    for c in range(8):
        b, s = c // 4, c % 4
        oc = results[c]["out"].astype(np.float32)  # (3, 512, 2025) rotated
        oc = np.roll(oc, 512 * s, axis=2)  # un-rotate p axis
        n_contig = _NPIX - 512 * s
        for u in range(3):
            if n_contig >= 512:
                full[b, u, 512 * s : 512 * s + 512] = oc[u]
            else:
                full[b, u, 512 * s :] = oc[u, :n_contig]
                full[b, u, : 512 - n_contig] = oc[u, n_contig:]
    out = np.empty((12, _NPIX, _NPIX), dtype=np.float32)
    for b in range(2):
        out[b * 6 + 0] = full[b, 0]
        out[b * 6 + 1] = full[b, 1]
        out[b * 6 + 2] = full[b, 0].T
        out[b * 6 + 3] = full[b, 2]
        out[b * 6 + 4] = full[b, 1].T
        out[b * 6 + 5] = full[b, 2].T
    return out.reshape(12, _NPIX, 45, 45)


def kernel(x, n):
    global _NC
    x = np.asarray(x, dtype=np.float32)
    assert int(n) == 3 and x.shape == (6, 512, 90, 90), (x.shape, n)
    from concourse.bass_utils import run_bass_kernel_spmd

    if _NC is None:
        _NC = _build_nc()
    res = run_bass_kernel_spmd(_NC, _core_inputs(x), core_ids=list(range(8)))
    return _gather(res.results)
